# revision 1
# baseline (speedup 1.0000x reference)
"""Binarized 1D convolution (K=5, Cin=Cout=256, SAME padding) + bias + ReLU
on 8 Trainium2 NeuronCores, data-parallel over the batch dimension.

Full inputs in, full output out:
  x: [64, 4096, 256] f32, W: [5, 256, 256] f32, b: [256] f32
  out[n, l, co] = relu(b[co] + sum_{k,ci} x[n, l+k-2, ci] * sign(W[k, ci, co]))

Per-core plan (8 batch rows each, identical SPMD program):
  - Activations flow through the PE as float32r (full 4-byte storage, PE
    rounds internally; ~1e-4 relative error, 4x faster than plain f32
    matmuls). Binarized weights are exactly representable.
  - Per 1024-l chunk: DMA x[l, ci] naturally, PE-transpose 128x128 blocks
    (identity matmul) into a [ci, l] strip with 2-column halos on each side;
    halos are stitched from the neighboring chunks (zeros at row edges for
    SAME padding).
  - Conv as 10 accumulating matmuls per 128-l output tile: lhsT = shifted
    [ci=128, l=128] strip window (stationary), rhs = sign(W[k]) [ci=128,
    co=256] (moving), PSUM-accumulated over k in 0..4 and both ci halves.
  - Bias is broadcast once via a K=1 matmul into SBUF, added per-tile on the
    vector engine (PSUM -> SBUF), ReLU in place on the scalar engine,
    1 MB coalesced stores.
  - Software pipeline: transposes run one chunk ahead of the matmuls; the
    first/last chunks are split small to fill and drain the pipeline fast.
"""

import numpy as np

B, L, CIN, COUT, KW = 64, 4096, 256, 256, 5
N_CORES = 8
B_PER_CORE = B // N_CORES
P = 128
CHUNK = 1024  # l positions per load/store DMA
LA = 1  # transpose lookahead (chunks emitted ahead of their matmuls)

_CACHE = {}


def _build():
    import concourse.bass as bass
    import concourse.mybir as mybir
    import concourse.tile as tile
    from concourse import bacc
    from concourse.masks import make_identity

    f32 = mybir.dt.float32
    f32r = mybir.dt.float32r
    u32 = mybir.dt.uint32

    nc = bacc.Bacc("TRN2", target_bir_lowering=False, debug=False)
    # x and b are declared float32r: identical bytes/numpy view as float32,
    # but walrus requires f32r matmul operands to be produced as f32r.
    x_d = nc.dram_tensor("x", (B_PER_CORE, L, CIN), f32r, kind="ExternalInput")
    w_d = nc.dram_tensor("W", (KW, CIN, COUT), f32, kind="ExternalInput")
    b_d = nc.dram_tensor("b", (1, COUT), f32r, kind="ExternalInput")
    out_d = nc.dram_tensor("out", (B_PER_CORE, L, COUT), f32, kind="ExternalOutput")

    NBLK = CHUNK // P  # max 128-l blocks per chunk

    with tile.TileContext(nc) as tc:
        with (
            tc.tile_pool(name="const", bufs=1) as const_pool,
            tc.tile_pool(name="wb", bufs=1) as wb_pool,
            tc.tile_pool(name="xc", bufs=6) as xc_pool,
            tc.tile_pool(name="strip", bufs=7) as strip_pool,
            tc.tile_pool(name="ow", bufs=4) as ow_pool,
            tc.tile_pool(name="pt", bufs=3, space=bass.MemorySpace.PSUM) as pt_pool,
            tc.tile_pool(name="po", bufs=5, space=bass.MemorySpace.PSUM) as po_pool,
        ):
            ident_f32 = const_pool.tile([P, P], f32)
            make_identity(nc, ident_f32[:])
            ident = const_pool.tile([P, P], f32r)
            nc.vector.tensor_copy(ident[:], ident_f32[:])

            def setup_bias():
                ones_f32 = const_pool.tile([1, P], f32)
                nc.gpsimd.memset(ones_f32[:], 1.0)
                ones = const_pool.tile([1, P], f32r)
                nc.vector.tensor_copy(ones[:], ones_f32[:])
                brow = const_pool.tile([1, COUT], f32r)
                nc.sync.dma_start(brow[:], b_d.ap()[:])
                # bias broadcast to all partitions via a one-time K=1 matmul
                bias_bc = const_pool.tile([P, COUT], f32)
                pb = pt_pool.tile([P, COUT], f32, tag="pt")
                nc.tensor.matmul(pb[:], ones[:], brow[:], start=True, stop=True)
                nc.vector.tensor_copy(bias_bc[:], pb[:])
                return bias_bc, ones, brow

            # Binarized weights: layout [ci=128, (k, ciT), co], loaded and
            # sign-binarized in two halves so the first matmuls start sooner.
            wsrc = w_d.ap().rearrange("k (t p) c -> p (k t) c", p=P)
            wb = {}

            def setup_weights_tap(k):
                wraw_k = wb_pool.tile([P, 2, COUT], f32, tag=f"wraw{k}")
                nc.sync.dma_start(wraw_k[:], wsrc[:, 2 * k : 2 * k + 2, :])
                wb_k = wb_pool.tile([P, 2, COUT], f32r, tag=f"wb{k}")
                nc.scalar.sign(wb_k[:], wraw_k[:])
                for ciT in range(2):
                    wb[(k, ciT)] = wb_k[:, ciT, :]

            # Per-chunk strips: [128 ci, 2 ci-halves, clen+6 cols], col j of
            # chunk c0 holds l = c0 - 2 + j. Leading halo (2 cols) comes from
            # the previous strip (memset at row start); trailing halo (2
            # cols) is stitched in by the NEXT chunk's first transposed block
            # (memset at row end).
            SW = CHUNK + 6

            def transpose_chunk(strip, strip_prev, prev_clen, r, c0, clen):
                nblk = clen // P
                xc = xc_pool.tile([P, NBLK, CIN], f32r, tag="xc")
                nc.sync.dma_start(
                    xc[:, :nblk, :],
                    x_d.ap()[r, c0 : c0 + clen, :].rearrange(
                        "(n p) c -> p n c", p=P
                    ),
                )
                if strip_prev is not None:
                    # leading halo (l = c0-2, c0-1) from the previous strip
                    for ciT in range(2):
                        nc.vector.tensor_copy(
                            strip[:, ciT, 0:2],
                            strip_prev[:, ciT, prev_clen : prev_clen + 2],
                        )
                for i in range(nblk):
                    for ciT in range(2):
                        pt = pt_pool.tile([P, P], f32r, tag="pt")
                        nc.tensor.transpose(
                            pt[:], xc[:, i, ciT * P : (ciT + 1) * P], ident[:]
                        )
                        col = 2 + i * P
                        nc.vector.tensor_copy(
                            strip[:, ciT, col : col + P], pt[:]
                        )
                        if i == 0 and strip_prev is not None:
                            # trailing halo of the previous strip
                            nc.vector.tensor_copy(
                                strip_prev[
                                    :, ciT, 2 + prev_clen : 4 + prev_clen
                                ],
                                pt[:, 0:2],
                            )

            def matmul_chunk(strip, r, c0, clen, last_chunk=False):
                nblk = clen // P
                ow = ow_pool.tile([P, NBLK, COUT], f32, tag="ow")
                for i in range(nblk):
                    po = po_pool.tile([P, COUT], f32, tag="po")
                    # tail variant: bias via K=1 PE matmul so ACT can ReLU
                    # straight from PSUM -- skips the DVE hop in the drain
                    pe_bias = last_chunk and i == nblk - 1
                    if pe_bias:
                        nc.tensor.matmul(
                            po[:], ones_r[:], brow_r[:], start=True, stop=False
                        )
                    # accumulate in wb-slice order: the first half only needs
                    # the first W-load+sign half, so startup matmuls begin
                    # before the second half lands
                    for idx in range(2 * KW):
                        k, ciT = idx // 2, idx % 2
                        nc.tensor.matmul(
                            po[:],
                            strip[:, ciT, i * P + k : i * P + k + P],
                            wb[(k, ciT)],
                            start=(idx == 0 and not pe_bias),
                            stop=(idx == 2 * KW - 1),
                        )
                    if pe_bias:
                        nc.scalar.activation(
                            ow[:, i, :], po[:], mybir.ActivationFunctionType.Relu
                        )
                    else:
                        # bias add on DVE, then ReLU in place on ACT
                        nc.vector.scalar_tensor_tensor(
                            ow[:, i, :],
                            po[:],
                            0.0,
                            bias_bc[:],
                            mybir.AluOpType.add,
                            mybir.AluOpType.add,
                        )
                        nc.scalar.activation(
                            ow[:, i, :],
                            ow[:, i, :],
                            mybir.ActivationFunctionType.Relu,
                        )
                nc.sync.dma_start(
                    out_d.ap()[r, c0 : c0 + clen, :].rearrange(
                        "(n p) c -> p n c", p=P
                    ),
                    ow[:, :nblk, :],
                )

            # Chunk list: 1024-l chunks, with the global first/last split
            # small so the pipeline fills and drains quickly.
            chunks = []
            for r in range(B_PER_CORE):
                sizes = [CHUNK] * (L // CHUNK)
                if r == 0:
                    sizes = [CHUNK // 4, CHUNK // 4, CHUNK // 2] + sizes[1:]
                if r == B_PER_CORE - 1:
                    sizes = sizes[:-1] + [CHUNK // 2, CHUNK // 4, CHUNK // 8, CHUNK // 8]
                c0 = 0
                for s in sizes:
                    chunks.append((r, c0, s))
                    c0 += s

            def new_strip(r, c0, clen):
                strip = strip_pool.tile([P, 2, SW], f32r, tag="strip")
                # SAME-padding zeros at row edges (uint32 view: gpsimd
                # memset cannot encode f32r directly)
                if c0 == 0:
                    for ciT in range(2):
                        nc.gpsimd.memset(strip[:, ciT, 0:2].bitcast(u32), 0)
                if c0 + clen == L:
                    for ciT in range(2):
                        nc.gpsimd.memset(
                            strip[:, ciT, 2 + clen : 4 + clen].bitcast(u32), 0
                        )
                return strip

            def emit_transpose(n):
                rn, cn, sn = chunks[n]
                strips[n] = new_strip(rn, cn, sn)
                prev = strips.get(n - 1) if cn != 0 else None
                prev_clen = chunks[n - 1][2] if n > 0 else 0
                transpose_chunk(strips[n], prev, prev_clen, rn, cn, sn)

            strips = {}
            for n in range(min(LA, len(chunks))):
                emit_transpose(n)
            for _k in range(KW):
                setup_weights_tap(_k)
            bias_bc, ones_r, brow_r = setup_bias()
            for n in range(len(chunks)):
                if n + LA < len(chunks):
                    emit_transpose(n + LA)
                matmul_chunk(
                    strips[n], *chunks[n], last_chunk=(n == len(chunks) - 1)
                )
                del strips[n]

    nc.compile()
    return nc


def _get_nc():
    if "nc" not in _CACHE:
        _CACHE["nc"] = _build()
    return _CACHE["nc"]


def kernel(x: np.ndarray, W: np.ndarray, b: np.ndarray) -> np.ndarray:
    from concourse import bass_utils

    nc = _get_nc()
    x = np.ascontiguousarray(x, dtype=np.float32)
    W = np.ascontiguousarray(W, dtype=np.float32)
    b2 = np.ascontiguousarray(b, dtype=np.float32).reshape(1, COUT)
    in_maps = [
        {
            "x": x[i * B_PER_CORE : (i + 1) * B_PER_CORE],
            "W": W,
            "b": b2,
        }
        for i in range(N_CORES)
    ]
    res = bass_utils.run_bass_kernel_spmd(nc, in_maps, core_ids=list(range(N_CORES)))
    return np.concatenate([res.results[i]["out"] for i in range(N_CORES)], axis=0)



# revision 3
# speedup vs baseline: 1.4912x; 1.4912x over previous
"""Binarized 1D convolution (K=5, Cin=Cout=256, SAME padding) + bias + ReLU
on 8 Trainium2 NeuronCores, data-parallel over the batch dimension.

Full inputs in, full output out:
  x: [64, 4096, 256] f32, W: [5, 256, 256] f32, b: [256] f32
  out[n, l, co] = relu(b[co] + sum_{k,ci} x[n, l+k-2, ci] * sign(W[k, ci, co]))

Per-core plan (8 batch rows each, identical SPMD program):
  - The conv runs on the PE in fp8e4 DoubleRow mode: each matmul contracts
    K=256 (both 128-ci halves as the DoubleRow pair) and streams output
    columns at 0.5 cycles/col -- 4x the f32r MAC rate. Since raw e4m3
    quantization of x costs ~2.6e-2 relative error (over the 2e-2 budget),
    x is split into two fp8 streams a = fp8(x), e = fp8(x - a); conv(a) +
    conv(e) restores ~7.5e-4 relative error at half the f32r PE time.
    Binarized weights are exact in fp8.
  - Per 1024-l chunk: DMA x[l, ci] naturally, PE-transpose 128x128 blocks
    (identity matmul) in groups of 4 into a single PSUM bank [128, 512]
    (one accumulation group: start on the first write, stop on the last).
    ACT quantizes the bank to the a-strip (fp8), DVE computes the e-strip
    via (xT - a) in one 512-col pass each: wide ops amortize the per-
    instruction SBUF/PSUM access latency.
  - Strips are [ci=128, 2 (ci-half), clen+6] fp8 with 2-col halos stitched
    from neighboring chunks (zeros at row edges for SAME padding).
  - Conv per PSUM output bank [l=128, 2, co=256] (two 128-l tiles, one
    accumulation group): 10 DoubleRow matmuls per tile, lhsT = shifted
    strip window [ci, 2, l] (stationary), rhs = sign(W[k]) [ci, 2, co].
  - Bias+ReLU drain alternates engines to keep everything under the DMA
    roofline (67 MB/core at 360 GB/s ~= 186 us): 1/3 of banks get bias via
    a K=1 bf16 PE matmul opening the accumulation group (ACT then ReLUs
    straight from PSUM), 2/3 get bias on the DVE (PSUM->SBUF) with an
    in-place ACT ReLU.
  - Software pipeline: transposes run one chunk ahead of the matmuls; the
    first/last chunks are split small to fill and drain the pipeline fast.
"""

import numpy as np

B, L, CIN, COUT, KW = 64, 4096, 256, 256, 5
N_CORES = 8
B_PER_CORE = B // N_CORES
P = 128
CHUNK = 1024  # l positions per load/store DMA
LA = 1  # transpose lookahead (chunks emitted ahead of their matmuls)
GRP = 4  # 128-l transpose blocks per PSUM bank / quantize instruction
BIAS_PE_MOD = 3  # 1 of every BIAS_PE_MOD output banks does bias on the PE

_CACHE = {}


def _build():
    import concourse.bass as bass
    import concourse.mybir as mybir
    import concourse.tile as tile
    from concourse import bacc
    from concourse.masks import make_identity

    f32 = mybir.dt.float32
    f32r = mybir.dt.float32r
    bf16 = mybir.dt.bfloat16
    fp8 = mybir.dt.float8e4
    u8 = mybir.dt.uint8
    DR = mybir.MatmulPerfMode.DoubleRow

    nc = bacc.Bacc("TRN2", target_bir_lowering=False, debug=False)
    x_d = nc.dram_tensor("x", (B_PER_CORE, L, CIN), f32r, kind="ExternalInput")
    w_d = nc.dram_tensor("W", (KW, CIN, COUT), f32, kind="ExternalInput")
    b_d = nc.dram_tensor("b", (1, COUT), f32r, kind="ExternalInput")
    out_d = nc.dram_tensor("out", (B_PER_CORE, L, COUT), f32, kind="ExternalOutput")

    NBLK = CHUNK // P  # max 128-l blocks per chunk

    with tile.TileContext(nc) as tc:
        with (
            tc.tile_pool(name="const", bufs=1) as const_pool,
            tc.tile_pool(name="wb", bufs=1) as wb_pool,
            tc.tile_pool(name="xc", bufs=6) as xc_pool,
            tc.tile_pool(name="stripa", bufs=5) as stripa_pool,
            tc.tile_pool(name="stripe", bufs=5) as stripe_pool,
            tc.tile_pool(name="ow", bufs=4) as ow_pool,
            tc.tile_pool(name="pt", bufs=4, space=bass.MemorySpace.PSUM) as pt_pool,
            tc.tile_pool(name="po", bufs=3, space=bass.MemorySpace.PSUM) as po_pool,
        ):
            ident_f32 = const_pool.tile([P, P], f32)
            make_identity(nc, ident_f32[:])
            ident = const_pool.tile([P, P], f32r)
            nc.vector.tensor_copy(ident[:], ident_f32[:])

            def setup_bias():
                ones_f32 = const_pool.tile([1, P], f32)
                nc.gpsimd.memset(ones_f32[:], 1.0)
                ones = const_pool.tile([1, P], f32r)
                nc.vector.tensor_copy(ones[:], ones_f32[:])
                brow = const_pool.tile([1, COUT], f32r)
                nc.sync.dma_start(brow[:], b_d.ap()[:])
                # bias broadcast to all partitions via a one-time K=1 matmul,
                # then duplicated into both 256-col halves of an output bank
                bias_bc2 = const_pool.tile([P, 2, COUT], f32)
                pb = pt_pool.tile([P, COUT], f32, tag="pt")
                nc.tensor.matmul(pb[:], ones[:], brow[:], start=True, stop=True)
                for t in range(2):
                    nc.vector.tensor_copy(bias_bc2[:, t, :], pb[:])
                # bf16 copies for the PE-side bias matmul
                ones_b = const_pool.tile([1, P], bf16)
                nc.vector.tensor_copy(ones_b[:], ones_f32[:])
                brow2_b = const_pool.tile([1, 2, COUT], bf16)
                for t in range(2):
                    nc.vector.tensor_copy(brow2_b[:, t, :], brow[:])
                return bias_bc2, ones_b, brow2_b

            # Binarized weights in fp8: layout [ci=128, (k, ciT), co]; the
            # ciT pair of each tap is the DoubleRow slot dim of the rhs.
            wsrc = w_d.ap().rearrange("k (t p) c -> p (k t) c", p=P)
            wb = {}

            def setup_weights_tap(k):
                wraw_k = wb_pool.tile([P, 2, COUT], f32, tag=f"wraw{k}")
                nc.sync.dma_start(wraw_k[:], wsrc[:, 2 * k : 2 * k + 2, :])
                wb_k = wb_pool.tile([P, 2, COUT], fp8, tag=f"wb{k}")
                nc.scalar.sign(wb_k[:], wraw_k[:])
                wb[k] = wb_k

            # Per-chunk strips: [128 ci, 2 ci-halves, clen+6 cols], col j of
            # chunk c0 holds l = c0 - 2 + j. Leading halo (2 cols) comes from
            # the previous strip (memset at row start); trailing halo (2
            # cols) is stitched from the NEXT chunk's first quantized cols
            # (memset at row end). Width padded to a multiple of 16: the
            # dual-fp8 Ldweights ISA check requires the slot-plane stride to
            # be 16-byte aligned (s3_lw_dual_fp8_restrictions).
            SW = CHUNK + 16

            def transpose_chunk(sa, se, prev, prev_clen, r, c0, clen):
                nblk = clen // P
                xc = xc_pool.tile([P, NBLK, CIN], f32r, tag="xc")
                nc.sync.dma_start(
                    xc[:, :nblk, :],
                    x_d.ap()[r, c0 : c0 + clen, :].rearrange(
                        "(n p) c -> p n c", p=P
                    ),
                )
                if prev is not None:
                    # leading halo (l = c0-2, c0-1) from the previous strip
                    pa, pe_ = prev
                    nc.gpsimd.tensor_copy(
                        sa[:, :, 0:2], pa[:, :, prev_clen : prev_clen + 2]
                    )
                    nc.gpsimd.tensor_copy(
                        se[:, :, 0:2], pe_[:, :, prev_clen : prev_clen + 2]
                    )
                for ciT in range(2):
                    for g0 in range(0, nblk, GRP):
                        gn = min(GRP, nblk - g0)
                        pt = pt_pool.tile([P, GRP * P], f32r, tag="pt")
                        for j in range(gn):
                            nc.tensor.matmul(
                                pt[:, j * P : (j + 1) * P],
                                xc[:, g0 + j, ciT * P : (ciT + 1) * P],
                                ident[:],
                                is_transpose=True,
                                start=(j == 0),
                                stop=(j == gn - 1),
                            )
                        col = 2 + g0 * P
                        ncols = gn * P
                        pt_f = pt[:, :ncols].bitcast(f32)
                        nc.scalar.activation(
                            sa[:, ciT, col : col + ncols],
                            pt_f,
                            mybir.ActivationFunctionType.Copy,
                        )
                        nc.vector.scalar_tensor_tensor(
                            se[:, ciT, col : col + ncols],
                            pt_f,
                            0.0,
                            sa[:, ciT, col : col + ncols],
                            mybir.AluOpType.add,
                            mybir.AluOpType.subtract,
                        )
                if prev is not None:
                    # trailing halo of the previous strip (l = c0, c0+1)
                    pa, pe_ = prev
                    nc.gpsimd.tensor_copy(
                        pa[:, :, 2 + prev_clen : 4 + prev_clen], sa[:, :, 2:4]
                    )
                    nc.gpsimd.tensor_copy(
                        pe_[:, :, 2 + prev_clen : 4 + prev_clen], se[:, :, 2:4]
                    )

            bias_ctr = [0]

            def matmul_chunk(sa, se, r, c0, clen):
                nblk = clen // P
                ow = ow_pool.tile([P, NBLK, COUT], f32, tag="ow")
                for t0 in range(0, nblk, 2):
                    nt = min(2, nblk - t0)
                    po = po_pool.tile([P, 2, COUT], f32, tag="po")
                    pe_bias = bias_ctr[0] % BIAS_PE_MOD == 0
                    bias_ctr[0] += 1
                    if pe_bias:
                        # K=1 bf16 matmul opens the accumulation group with
                        # the bias already in PSUM
                        nc.tensor.matmul(
                            po[:, :nt, :],
                            ones_b[:],
                            brow2_b[:, :nt, :],
                            start=True,
                            stop=False,
                        )
                    n_mm = nt * 2 * KW
                    idx = 0
                    for t in range(nt):
                        for strip in (sa, se):
                            for k in range(KW):
                                off = (t0 + t) * P + k
                                nc.tensor.matmul(
                                    po[:, t, :],
                                    strip[:, :, off : off + P],
                                    wb[k][:],
                                    start=(idx == 0 and not pe_bias),
                                    stop=(idx == n_mm - 1),
                                    perf_mode=DR,
                                )
                                idx += 1
                    if pe_bias:
                        nc.scalar.activation(
                            ow[:, t0 : t0 + nt, :],
                            po[:, :nt, :],
                            mybir.ActivationFunctionType.Relu,
                        )
                    else:
                        nc.vector.scalar_tensor_tensor(
                            ow[:, t0 : t0 + nt, :],
                            po[:, :nt, :],
                            0.0,
                            bias_bc2[:, :nt, :],
                            mybir.AluOpType.add,
                            mybir.AluOpType.add,
                        )
                        nc.scalar.activation(
                            ow[:, t0 : t0 + nt, :],
                            ow[:, t0 : t0 + nt, :],
                            mybir.ActivationFunctionType.Relu,
                        )
                nc.sync.dma_start(
                    out_d.ap()[r, c0 : c0 + clen, :].rearrange(
                        "(n p) c -> p n c", p=P
                    ),
                    ow[:, :nblk, :],
                )

            # Chunk list: 1024-l chunks, with the global first/last split
            # small so the pipeline fills and drains quickly.
            chunks = []
            for r in range(B_PER_CORE):
                sizes = [CHUNK] * (L // CHUNK)
                if r == 0:
                    sizes = [CHUNK // 4, CHUNK // 4, CHUNK // 2] + sizes[1:]
                if r == B_PER_CORE - 1:
                    sizes = sizes[:-1] + [CHUNK // 2, CHUNK // 4, CHUNK // 8, CHUNK // 8]
                c0 = 0
                for s in sizes:
                    chunks.append((r, c0, s))
                    c0 += s

            def new_strips(r, c0, clen):
                sa = stripa_pool.tile([P, 2, SW], fp8, tag="sa")
                se = stripe_pool.tile([P, 2, SW], fp8, tag="se")
                # SAME-padding zeros at row edges
                if c0 == 0:
                    nc.gpsimd.memset(sa[:, :, 0:2].bitcast(u8), 0)
                    nc.gpsimd.memset(se[:, :, 0:2].bitcast(u8), 0)
                if c0 + clen == L:
                    nc.gpsimd.memset(sa[:, :, 2 + clen : 4 + clen].bitcast(u8), 0)
                    nc.gpsimd.memset(se[:, :, 2 + clen : 4 + clen].bitcast(u8), 0)
                return sa, se

            def emit_transpose(n):
                rn, cn, sn = chunks[n]
                strips[n] = new_strips(rn, cn, sn)
                prev = strips.get(n - 1) if cn != 0 else None
                prev_clen = chunks[n - 1][2] if n > 0 else 0
                transpose_chunk(*strips[n], prev, prev_clen, rn, cn, sn)

            strips = {}
            for n in range(min(LA, len(chunks))):
                emit_transpose(n)
            for _k in range(KW):
                setup_weights_tap(_k)
            bias_bc2, ones_b, brow2_b = setup_bias()
            for n in range(len(chunks)):
                if n + LA < len(chunks):
                    emit_transpose(n + LA)
                matmul_chunk(*strips[n], *chunks[n])
                del strips[n]

    nc.compile()
    return nc


def _get_nc():
    if "nc" not in _CACHE:
        _CACHE["nc"] = _build()
    return _CACHE["nc"]


def kernel(x: np.ndarray, W: np.ndarray, b: np.ndarray) -> np.ndarray:
    from concourse import bass_utils

    nc = _get_nc()
    x = np.ascontiguousarray(x, dtype=np.float32)
    W = np.ascontiguousarray(W, dtype=np.float32)
    b2 = np.ascontiguousarray(b, dtype=np.float32).reshape(1, COUT)
    in_maps = [
        {
            "x": x[i * B_PER_CORE : (i + 1) * B_PER_CORE],
            "W": W,
            "b": b2,
        }
        for i in range(N_CORES)
    ]
    res = bass_utils.run_bass_kernel_spmd(nc, in_maps, core_ids=list(range(N_CORES)))
    return np.concatenate([res.results[i]["out"] for i in range(N_CORES)], axis=0)


# revision 12
# speedup vs baseline: 1.6838x; 1.1292x over previous
"""Binarized 1D convolution (K=5, Cin=Cout=256, SAME padding) + bias + ReLU
on 8 Trainium2 NeuronCores, data-parallel over the batch dimension.

Full inputs in, full output out:
  x: [64, 4096, 256] f32, W: [5, 256, 256] f32, b: [256] f32
  out[n, l, co] = relu(b[co] + sum_{k,ci} x[n, l+k-2, ci] * sign(W[k, ci, co]))

Per-core plan (8 batch rows each, identical SPMD program):
  - The conv runs on the PE in fp8e4 DoubleRow mode: each matmul contracts
    K=256 (both 128-ci halves as the DoubleRow pair) and streams output
    columns at 0.5 cycles/col -- 4x the f32r MAC rate. Since raw e4m3
    quantization of x costs ~2.6e-2 relative error (over the 2e-2 budget),
    x is split into two fp8 streams a = fp8(x), e = fp8(x - a); conv(a) +
    conv(e) restores ~7.5e-4 relative error at half the f32r PE time.
    Binarized weights are exact in fp8.
  - Per 1024-l chunk: DMA x[l, ci] naturally, PE-transpose 128x128 blocks
    (identity matmul) in groups of 4 into a single PSUM bank [128, 512]
    (one accumulation group: start on the first write, stop on the last).
    ACT quantizes the bank to the a-strip (fp8), DVE computes the e-strip
    via (xT - a) in one 512-col pass each: wide ops amortize the per-
    instruction SBUF/PSUM access latency.
  - Strips are [ci=128, 2 (ci-half), clen+6] fp8 with 2-col halos stitched
    from neighboring chunks (zeros at row edges for SAME padding).
  - Conv per PSUM output bank [l=128, 2, co=256] (two 128-l tiles, one
    accumulation group): 10 DoubleRow matmuls per tile, lhsT = shifted
    strip window [ci, 2, l] (stationary), rhs = sign(W[k]) [ci, 2, co].
  - Bias+ReLU drain alternates engines to keep everything under the DMA
    roofline (67 MB/core at 360 GB/s ~= 186 us): 1/3 of banks get bias via
    a K=1 bf16 PE matmul opening the accumulation group (ACT then ReLUs
    straight from PSUM), 2/3 get bias on the DVE (PSUM->SBUF) with an
    in-place ACT ReLU.
  - Software pipeline: transposes run one chunk ahead of the matmuls; the
    first/last chunks are split small to fill and drain the pipeline fast.
"""

import os
import numpy as np

B, L, CIN, COUT, KW = 64, 4096, 256, 256, 5
N_CORES = 8
B_PER_CORE = B // N_CORES
P = 128
CHUNK = 1024  # l positions per load/store DMA
LA = int(os.environ.get("K_LA", "2"))  # transpose lookahead (chunks)
GRP = int(os.environ.get("K_GRP", "8"))  # transpose blocks per pt tile
BIAS_PE_MOD = int(os.environ.get("K_BIAS_MOD", "1000000"))  # 1/N banks bias on PE
XC_BUFS = int(os.environ.get("K_XC_BUFS", "6"))
OW_BUFS = int(os.environ.get("K_OW_BUFS", "6"))
STRIP_BUFS = int(os.environ.get("K_STRIP_BUFS", "5"))
PT_BUFS = int(os.environ.get("K_PT_BUFS", "2"))
PO_BUFS = int(os.environ.get("K_PO_BUFS", "4"))
STORE_ENG = os.environ.get("K_STORE_ENG", "gpsimd")  # sync | gpsimd | vector
HALO_ENG = os.environ.get("K_HALO_ENG", "vector")  # gpsimd | vector

_CACHE = {}


def _build():
    import concourse.bass as bass
    import concourse.mybir as mybir
    import concourse.tile as tile
    from concourse import bacc
    from concourse.masks import make_identity

    f32 = mybir.dt.float32
    f32r = mybir.dt.float32r
    bf16 = mybir.dt.bfloat16
    fp8 = mybir.dt.float8e4
    u8 = mybir.dt.uint8
    DR = mybir.MatmulPerfMode.DoubleRow

    nc = bacc.Bacc("TRN2", target_bir_lowering=False, debug=False)
    x_d = nc.dram_tensor("x", (B_PER_CORE, L, CIN), f32r, kind="ExternalInput")
    w_d = nc.dram_tensor("W", (KW, CIN, COUT), f32, kind="ExternalInput")
    b_d = nc.dram_tensor("b", (1, COUT), f32r, kind="ExternalInput")
    out_d = nc.dram_tensor("out", (B_PER_CORE, L, COUT), f32, kind="ExternalOutput")

    NBLK = CHUNK // P  # max 128-l blocks per chunk

    with tile.TileContext(nc) as tc:
        with (
            tc.tile_pool(name="const", bufs=1) as const_pool,
            tc.tile_pool(name="wb", bufs=1) as wb_pool,
            tc.tile_pool(name="xc", bufs=XC_BUFS) as xc_pool,
            tc.tile_pool(name="stripa", bufs=STRIP_BUFS) as stripa_pool,
            tc.tile_pool(name="stripe", bufs=STRIP_BUFS) as stripe_pool,
            tc.tile_pool(name="ow", bufs=OW_BUFS) as ow_pool,
            tc.tile_pool(name="pt", bufs=PT_BUFS, space=bass.MemorySpace.PSUM) as pt_pool,
            tc.tile_pool(name="po", bufs=PO_BUFS, space=bass.MemorySpace.PSUM) as po_pool,
        ):
            ident_f32 = const_pool.tile([P, P], f32)
            make_identity(nc, ident_f32[:])
            ident = const_pool.tile([P, P], f32r)
            nc.vector.tensor_copy(ident[:], ident_f32[:])

            def setup_bias():
                ones_f32 = const_pool.tile([1, P], f32)
                nc.gpsimd.memset(ones_f32[:], 1.0)
                ones = const_pool.tile([1, P], f32r)
                nc.vector.tensor_copy(ones[:], ones_f32[:])
                brow = const_pool.tile([1, COUT], f32r)
                nc.sync.dma_start(brow[:], b_d.ap()[:])
                # bias broadcast to all partitions via a one-time K=1 matmul,
                # then duplicated into both 256-col halves of an output bank
                bias_bc2 = const_pool.tile([P, 2, COUT], f32)
                pb = pt_pool.tile([P, COUT], f32, tag="pt")
                nc.tensor.matmul(pb[:], ones[:], brow[:], start=True, stop=True)
                for t in range(2):
                    nc.vector.tensor_copy(bias_bc2[:, t, :], pb[:])
                # fp8 DoubleRow operands for the PE-side bias matmul: slot 0
                # carries ones x b, slot 1 is zeroed (DR sums both slots)
                ones_q = const_pool.tile([1, 2, P], fp8)
                onesq_f = const_pool.tile([1, 2, P], f32)
                nc.gpsimd.memset(onesq_f[:, 0, :], 1.0)
                nc.gpsimd.memset(onesq_f[:, 1, :], 0.0)
                nc.vector.tensor_copy(ones_q[:], onesq_f[:])
                brow_f = const_pool.tile([1, COUT], f32)
                nc.vector.tensor_copy(brow_f[:], brow[:].bitcast(f32))
                brow_q = const_pool.tile([1, 2, 2, COUT], fp8)
                for t in range(2):
                    nc.vector.tensor_copy(brow_q[:, 0, t, :], brow_f[:])
                nc.gpsimd.memset(brow_q[:, 1, :, :].bitcast(u8), 0)
                return bias_bc2, ones_q, brow_q

            # Binarized weights in fp8: layout [ci=128, (k, ciT), co]; the
            # ciT pair of each tap is the DoubleRow slot dim of the rhs.
            wsrc = w_d.ap().rearrange("k (t p) c -> p (k t) c", p=P)
            wb = {}

            def setup_weights_tap(k):
                wraw_k = wb_pool.tile([P, 2, COUT], f32, tag=f"wraw{k}")
                nc.sync.dma_start(wraw_k[:], wsrc[:, 2 * k : 2 * k + 2, :])
                wb_k = wb_pool.tile([P, 2, COUT], fp8, tag=f"wb{k}")
                nc.scalar.sign(wb_k[:], wraw_k[:])
                wb[k] = wb_k

            # Per-chunk strips: [128 ci, 2 ci-halves, clen+6 cols], col j of
            # chunk c0 holds l = c0 - 2 + j. Leading halo (2 cols) comes from
            # the previous strip (memset at row start); trailing halo (2
            # cols) is stitched from the NEXT chunk's first quantized cols
            # (memset at row end). Width padded to a multiple of 16: the
            # dual-fp8 Ldweights ISA check requires the slot-plane stride to
            # be 16-byte aligned (s3_lw_dual_fp8_restrictions).
            SW = CHUNK + 16

            def transpose_chunk(sa, se, prev, prev_clen, r, c0, clen):
                nblk = clen // P
                xc = xc_pool.tile([P, NBLK, CIN], f32r, tag="xc")
                nc.sync.dma_start(
                    xc[:, :nblk, :],
                    x_d.ap()[r, c0 : c0 + clen, :].rearrange(
                        "(n p) c -> p n c", p=P
                    ),
                )
                if prev is not None:
                    # leading halo (l = c0-2, c0-1) from the previous strip
                    pa, pe_ = prev
                    getattr(nc, HALO_ENG).tensor_copy(
                        sa[:, :, 0:2], pa[:, :, prev_clen : prev_clen + 2]
                    )
                    getattr(nc, HALO_ENG).tensor_copy(
                        se[:, :, 0:2], pe_[:, :, prev_clen : prev_clen + 2]
                    )
                for ciT in range(2):
                    for g0 in range(0, nblk, GRP):
                        gn = min(GRP, nblk - g0)
                        pt = pt_pool.tile([P, GRP * P], f32r, tag="pt")
                        for j in range(gn):
                            # PSUM accumulation groups are per 2KB zero
                            # region (4 x 128 f32 cols): open/close one group
                            # per bank within the tile
                            nc.tensor.matmul(
                                pt[:, j * P : (j + 1) * P],
                                xc[:, g0 + j, ciT * P : (ciT + 1) * P],
                                ident[:],
                                is_transpose=True,
                                start=(j % 4 == 0),
                                stop=(j % 4 == 3 or j == gn - 1),
                            )
                        col = 2 + g0 * P
                        ncols = gn * P
                        pt_f = pt[:, :ncols].bitcast(f32)
                        nc.scalar.activation(
                            sa[:, ciT, col : col + ncols],
                            pt_f,
                            mybir.ActivationFunctionType.Copy,
                        )
                        nc.vector.scalar_tensor_tensor(
                            se[:, ciT, col : col + ncols],
                            pt_f,
                            0.0,
                            sa[:, ciT, col : col + ncols],
                            mybir.AluOpType.add,
                            mybir.AluOpType.subtract,
                        )
                if prev is not None:
                    # trailing halo of the previous strip (l = c0, c0+1)
                    pa, pe_ = prev
                    getattr(nc, HALO_ENG).tensor_copy(
                        pa[:, :, 2 + prev_clen : 4 + prev_clen], sa[:, :, 2:4]
                    )
                    getattr(nc, HALO_ENG).tensor_copy(
                        pe_[:, :, 2 + prev_clen : 4 + prev_clen], se[:, :, 2:4]
                    )

            bias_ctr = [0]

            def matmul_chunk(sa, se, r, c0, clen):
                nblk = clen // P
                ow = ow_pool.tile([P, NBLK, COUT], f32, tag="ow")
                for t0 in range(0, nblk, 2):
                    nt = min(2, nblk - t0)
                    po = po_pool.tile([P, 2, COUT], f32, tag="po")
                    pe_bias = bias_ctr[0] % BIAS_PE_MOD == 0
                    bias_ctr[0] += 1
                    if pe_bias:
                        # K=1 fp8 DoubleRow matmul opens the accumulation
                        # group with the bias already in PSUM
                        nc.tensor.matmul(
                            po[:, :nt, :],
                            ones_q[:],
                            brow_q[:, :, :nt, :],
                            start=True,
                            stop=False,
                            perf_mode=DR,
                        )
                    n_mm = nt * 2 * KW
                    idx = 0
                    for t in range(nt):
                        for strip in (sa, se):
                            for k in range(KW):
                                off = (t0 + t) * P + k
                                nc.tensor.matmul(
                                    po[:, t, :],
                                    strip[:, :, off : off + P],
                                    wb[k][:],
                                    start=(idx == 0 and not pe_bias),
                                    stop=(idx == n_mm - 1),
                                    perf_mode=DR,
                                )
                                idx += 1
                    if pe_bias:
                        nc.scalar.activation(
                            ow[:, t0 : t0 + nt, :],
                            po[:, :nt, :],
                            mybir.ActivationFunctionType.Relu,
                        )
                    else:
                        nc.vector.scalar_tensor_tensor(
                            ow[:, t0 : t0 + nt, :],
                            po[:, :nt, :],
                            0.0,
                            bias_bc2[:, :nt, :],
                            mybir.AluOpType.add,
                            mybir.AluOpType.add,
                        )
                        nc.scalar.activation(
                            ow[:, t0 : t0 + nt, :],
                            ow[:, t0 : t0 + nt, :],
                            mybir.ActivationFunctionType.Relu,
                        )
                getattr(nc, STORE_ENG).dma_start(
                    out_d.ap()[r, c0 : c0 + clen, :].rearrange(
                        "(n p) c -> p n c", p=P
                    ),
                    ow[:, :nblk, :],
                )

            # Chunk list: 1024-l chunks, with the global first/last split
            # small so the pipeline fills and drains quickly.
            chunks = []
            for r in range(B_PER_CORE):
                sizes = [CHUNK] * (L // CHUNK)
                if r == 0:
                    sizes = [CHUNK // 4, CHUNK // 4, CHUNK // 2] + sizes[1:]
                if r == B_PER_CORE - 1:
                    sizes = sizes[:-1] + [CHUNK // 2, CHUNK // 4, CHUNK // 8, CHUNK // 8]
                c0 = 0
                for s in sizes:
                    chunks.append((r, c0, s))
                    c0 += s

            def new_strips(r, c0, clen):
                sa = stripa_pool.tile([P, 2, SW], fp8, tag="sa")
                se = stripe_pool.tile([P, 2, SW], fp8, tag="se")
                # SAME-padding zeros at row edges
                if c0 == 0:
                    nc.gpsimd.memset(sa[:, :, 0:2].bitcast(u8), 0)
                    nc.gpsimd.memset(se[:, :, 0:2].bitcast(u8), 0)
                if c0 + clen == L:
                    nc.gpsimd.memset(sa[:, :, 2 + clen : 4 + clen].bitcast(u8), 0)
                    nc.gpsimd.memset(se[:, :, 2 + clen : 4 + clen].bitcast(u8), 0)
                return sa, se

            def emit_transpose(n):
                rn, cn, sn = chunks[n]
                strips[n] = new_strips(rn, cn, sn)
                prev = strips.get(n - 1) if cn != 0 else None
                prev_clen = chunks[n - 1][2] if n > 0 else 0
                transpose_chunk(*strips[n], prev, prev_clen, rn, cn, sn)

            strips = {}
            for n in range(min(LA, len(chunks))):
                emit_transpose(n)
            for _k in range(KW):
                setup_weights_tap(_k)
            bias_bc2, ones_q, brow_q = setup_bias()
            for n in range(len(chunks)):
                if n + LA < len(chunks):
                    emit_transpose(n + LA)
                matmul_chunk(*strips[n], *chunks[n])
                del strips[n]

    nc.compile()
    return nc


def _get_nc():
    if "nc" not in _CACHE:
        _CACHE["nc"] = _build()
    return _CACHE["nc"]


def kernel(x: np.ndarray, W: np.ndarray, b: np.ndarray) -> np.ndarray:
    from concourse import bass_utils

    nc = _get_nc()
    x = np.ascontiguousarray(x, dtype=np.float32)
    W = np.ascontiguousarray(W, dtype=np.float32)
    b2 = np.ascontiguousarray(b, dtype=np.float32).reshape(1, COUT)
    in_maps = [
        {
            "x": x[i * B_PER_CORE : (i + 1) * B_PER_CORE],
            "W": W,
            "b": b2,
        }
        for i in range(N_CORES)
    ]
    res = bass_utils.run_bass_kernel_spmd(nc, in_maps, core_ids=list(range(N_CORES)))
    return np.concatenate([res.results[i]["out"] for i in range(N_CORES)], axis=0)


# revision 14
# speedup vs baseline: 1.6927x; 1.0053x over previous
"""Binarized 1D convolution (K=5, Cin=Cout=256, SAME padding) + bias + ReLU
on 8 Trainium2 NeuronCores, data-parallel over the batch dimension.

Full inputs in, full output out:
  x: [64, 4096, 256] f32, W: [5, 256, 256] f32, b: [256] f32
  out[n, l, co] = relu(b[co] + sum_{k,ci} x[n, l+k-2, ci] * sign(W[k, ci, co]))

Per-core plan (8 batch rows each, identical SPMD program):
  - The conv runs on the PE in fp8e4 DoubleRow mode: each matmul contracts
    K=256 (both 128-ci halves as the DoubleRow pair) and streams output
    columns at 0.5 cycles/col -- 4x the f32r MAC rate. Since raw e4m3
    quantization of x costs ~2.6e-2 relative error (over the 2e-2 budget),
    x is split into two fp8 streams a = fp8(x), e = fp8(x - a); conv(a) +
    conv(e) restores ~7.5e-4 relative error at half the f32r PE time.
    Binarized weights are exact in fp8.
  - Per 1024-l chunk: DMA x[l, ci] naturally, PE-transpose 128x128 blocks
    (identity matmul) in groups of 4 into a single PSUM bank [128, 512]
    (one accumulation group per 2KB zero region: start on the first write,
    stop on the last). ACT quantizes the bank to the a-strip (fp8), DVE
    computes the e-strip via (xT - a) in one 512-col pass each: wide ops
    amortize the ~125ns per-instruction SBUF/PSUM access latency. The
    sloppy ACT f32->fp8 rounding on real HW is harmless: e absorbs
    whatever a was.
  - Strips are [ci=128, 2 (ci-half), CHUNK+16] fp8 with 2-col halos
    stitched from neighboring chunks (zeros at row edges for SAME
    padding). Width is padded to a multiple of 16: the dual-fp8 Ldweights
    ISA check (s3_lw_dual_fp8_restrictions) requires a 16-byte-aligned
    slot-plane stride.
  - Conv per PSUM output bank [l=128, 2, co=256] (two 128-l tiles, one
    accumulation group): 10 DoubleRow matmuls per tile, lhsT = shifted
    strip window [ci, 2, l] (stationary), rhs = sign(W[k]) [ci, 2, co].
  - The kernel is DMA-roofline-bound (67 MB/core at 360 GB/s ~= 186 us),
    so the drain stays off the PE: DVE adds bias (PSUM->SBUF), ACT ReLUs
    in place, and the store DMAs issue from the Pool SWDGE queue so
    stores waiting on compute never block input loads queued on SP.
  - Software pipeline: transposes run two chunks ahead of the matmuls;
    the first/last chunks are split small to fill and drain fast.

  Engine budget per core (cost model): DMA 190us (bound), PE 178us
  (136.6 conv + 41 transposes), ACT ~145us, DVE ~170us, Pool ~65us.
"""

import numpy as np

B, L, CIN, COUT, KW = 64, 4096, 256, 256, 5
N_CORES = 8
B_PER_CORE = B // N_CORES
P = 128
CHUNK = 1024  # l positions per load/store DMA
LA = 2  # transpose lookahead (chunks emitted ahead of their matmuls)
GRP = 4  # 128-l transpose blocks per pt PSUM tile / quantize instruction
BIAS_PE_MOD = 1000000  # bias runs on the DVE (PE-bias variant disabled)
XC_BUFS = 6
OW_BUFS = 6
STRIP_BUFS = 5
PT_BUFS = 4
PO_BUFS = 4
STORE_ENG = "gpsimd"  # store DMAs on the Pool SWDGE queue: never block loads
HALO_ENG = "gpsimd"

_CACHE = {}


def _build():
    import concourse.bass as bass
    import concourse.mybir as mybir
    import concourse.tile as tile
    from concourse import bacc
    from concourse.masks import make_identity

    f32 = mybir.dt.float32
    f32r = mybir.dt.float32r
    bf16 = mybir.dt.bfloat16
    fp8 = mybir.dt.float8e4
    u8 = mybir.dt.uint8
    DR = mybir.MatmulPerfMode.DoubleRow

    nc = bacc.Bacc("TRN2", target_bir_lowering=False, debug=False)
    x_d = nc.dram_tensor("x", (B_PER_CORE, L, CIN), f32r, kind="ExternalInput")
    w_d = nc.dram_tensor("W", (KW, CIN, COUT), f32, kind="ExternalInput")
    b_d = nc.dram_tensor("b", (1, COUT), f32r, kind="ExternalInput")
    out_d = nc.dram_tensor("out", (B_PER_CORE, L, COUT), f32, kind="ExternalOutput")

    NBLK = CHUNK // P  # max 128-l blocks per chunk

    with tile.TileContext(nc) as tc:
        with (
            tc.tile_pool(name="const", bufs=1) as const_pool,
            tc.tile_pool(name="wb", bufs=1) as wb_pool,
            tc.tile_pool(name="xc", bufs=XC_BUFS) as xc_pool,
            tc.tile_pool(name="stripa", bufs=STRIP_BUFS) as stripa_pool,
            tc.tile_pool(name="stripe", bufs=STRIP_BUFS) as stripe_pool,
            tc.tile_pool(name="ow", bufs=OW_BUFS) as ow_pool,
            tc.tile_pool(name="pt", bufs=PT_BUFS, space=bass.MemorySpace.PSUM) as pt_pool,
            tc.tile_pool(name="po", bufs=PO_BUFS, space=bass.MemorySpace.PSUM) as po_pool,
        ):
            ident_f32 = const_pool.tile([P, P], f32)
            make_identity(nc, ident_f32[:])
            ident = const_pool.tile([P, P], f32r)
            nc.vector.tensor_copy(ident[:], ident_f32[:])

            def setup_bias():
                ones_f32 = const_pool.tile([1, P], f32)
                nc.gpsimd.memset(ones_f32[:], 1.0)
                ones = const_pool.tile([1, P], f32r)
                nc.vector.tensor_copy(ones[:], ones_f32[:])
                brow = const_pool.tile([1, COUT], f32r)
                nc.sync.dma_start(brow[:], b_d.ap()[:])
                # bias broadcast to all partitions via a one-time K=1 matmul,
                # then duplicated into both 256-col halves of an output bank
                bias_bc2 = const_pool.tile([P, 2, COUT], f32)
                pb = pt_pool.tile([P, COUT], f32, tag="pt")
                nc.tensor.matmul(pb[:], ones[:], brow[:], start=True, stop=True)
                for t in range(2):
                    nc.vector.tensor_copy(bias_bc2[:, t, :], pb[:])
                # fp8 DoubleRow operands for the PE-side bias matmul: slot 0
                # carries ones x b, slot 1 is zeroed (DR sums both slots)
                ones_q = const_pool.tile([1, 2, P], fp8)
                onesq_f = const_pool.tile([1, 2, P], f32)
                nc.gpsimd.memset(onesq_f[:, 0, :], 1.0)
                nc.gpsimd.memset(onesq_f[:, 1, :], 0.0)
                nc.vector.tensor_copy(ones_q[:], onesq_f[:])
                brow_f = const_pool.tile([1, COUT], f32)
                nc.vector.tensor_copy(brow_f[:], brow[:].bitcast(f32))
                brow_q = const_pool.tile([1, 2, 2, COUT], fp8)
                for t in range(2):
                    nc.vector.tensor_copy(brow_q[:, 0, t, :], brow_f[:])
                nc.gpsimd.memset(brow_q[:, 1, :, :].bitcast(u8), 0)
                return bias_bc2, ones_q, brow_q

            # Binarized weights in fp8: layout [ci=128, (k, ciT), co]; the
            # ciT pair of each tap is the DoubleRow slot dim of the rhs.
            wsrc = w_d.ap().rearrange("k (t p) c -> p (k t) c", p=P)
            wb = {}

            def setup_weights_tap(k):
                wraw_k = wb_pool.tile([P, 2, COUT], f32, tag=f"wraw{k}")
                nc.sync.dma_start(wraw_k[:], wsrc[:, 2 * k : 2 * k + 2, :])
                wb_k = wb_pool.tile([P, 2, COUT], fp8, tag=f"wb{k}")
                nc.scalar.sign(wb_k[:], wraw_k[:])
                wb[k] = wb_k

            # Per-chunk strips: [128 ci, 2 ci-halves, clen+6 cols], col j of
            # chunk c0 holds l = c0 - 2 + j. Leading halo (2 cols) comes from
            # the previous strip (memset at row start); trailing halo (2
            # cols) is stitched from the NEXT chunk's first quantized cols
            # (memset at row end). Width padded to a multiple of 16: the
            # dual-fp8 Ldweights ISA check requires the slot-plane stride to
            # be 16-byte aligned (s3_lw_dual_fp8_restrictions).
            SW = CHUNK + 16

            def transpose_chunk(sa, se, prev, prev_clen, r, c0, clen):
                nblk = clen // P
                xc = xc_pool.tile([P, NBLK, CIN], f32r, tag="xc")
                nc.sync.dma_start(
                    xc[:, :nblk, :],
                    x_d.ap()[r, c0 : c0 + clen, :].rearrange(
                        "(n p) c -> p n c", p=P
                    ),
                )
                if prev is not None:
                    # leading halo (l = c0-2, c0-1) from the previous strip
                    pa, pe_ = prev
                    getattr(nc, HALO_ENG).tensor_copy(
                        sa[:, :, 0:2], pa[:, :, prev_clen : prev_clen + 2]
                    )
                    getattr(nc, HALO_ENG).tensor_copy(
                        se[:, :, 0:2], pe_[:, :, prev_clen : prev_clen + 2]
                    )
                for ciT in range(2):
                    for g0 in range(0, nblk, GRP):
                        gn = min(GRP, nblk - g0)
                        pt = pt_pool.tile([P, GRP * P], f32r, tag="pt")
                        for j in range(gn):
                            # PSUM accumulation groups are per 2KB zero
                            # region (4 x 128 f32 cols): open/close one group
                            # per bank within the tile
                            nc.tensor.matmul(
                                pt[:, j * P : (j + 1) * P],
                                xc[:, g0 + j, ciT * P : (ciT + 1) * P],
                                ident[:],
                                is_transpose=True,
                                start=(j % 4 == 0),
                                stop=(j % 4 == 3 or j == gn - 1),
                            )
                        col = 2 + g0 * P
                        ncols = gn * P
                        pt_f = pt[:, :ncols].bitcast(f32)
                        nc.scalar.activation(
                            sa[:, ciT, col : col + ncols],
                            pt_f,
                            mybir.ActivationFunctionType.Copy,
                        )
                        nc.vector.scalar_tensor_tensor(
                            se[:, ciT, col : col + ncols],
                            pt_f,
                            0.0,
                            sa[:, ciT, col : col + ncols],
                            mybir.AluOpType.add,
                            mybir.AluOpType.subtract,
                        )
                if prev is not None:
                    # trailing halo of the previous strip (l = c0, c0+1)
                    pa, pe_ = prev
                    getattr(nc, HALO_ENG).tensor_copy(
                        pa[:, :, 2 + prev_clen : 4 + prev_clen], sa[:, :, 2:4]
                    )
                    getattr(nc, HALO_ENG).tensor_copy(
                        pe_[:, :, 2 + prev_clen : 4 + prev_clen], se[:, :, 2:4]
                    )

            bias_ctr = [0]

            def matmul_chunk(sa, se, r, c0, clen):
                nblk = clen // P
                ow = ow_pool.tile([P, NBLK, COUT], f32, tag="ow")
                for t0 in range(0, nblk, 2):
                    nt = min(2, nblk - t0)
                    po = po_pool.tile([P, 2, COUT], f32, tag="po")
                    pe_bias = bias_ctr[0] % BIAS_PE_MOD == 0
                    bias_ctr[0] += 1
                    if pe_bias:
                        # K=1 fp8 DoubleRow matmul opens the accumulation
                        # group with the bias already in PSUM
                        nc.tensor.matmul(
                            po[:, :nt, :],
                            ones_q[:],
                            brow_q[:, :, :nt, :],
                            start=True,
                            stop=False,
                            perf_mode=DR,
                        )
                    n_mm = nt * 2 * KW
                    idx = 0
                    for t in range(nt):
                        for strip in (sa, se):
                            for k in range(KW):
                                off = (t0 + t) * P + k
                                nc.tensor.matmul(
                                    po[:, t, :],
                                    strip[:, :, off : off + P],
                                    wb[k][:],
                                    start=(idx == 0 and not pe_bias),
                                    stop=(idx == n_mm - 1),
                                    perf_mode=DR,
                                )
                                idx += 1
                    if pe_bias:
                        nc.scalar.activation(
                            ow[:, t0 : t0 + nt, :],
                            po[:, :nt, :],
                            mybir.ActivationFunctionType.Relu,
                        )
                    else:
                        nc.vector.scalar_tensor_tensor(
                            ow[:, t0 : t0 + nt, :],
                            po[:, :nt, :],
                            0.0,
                            bias_bc2[:, :nt, :],
                            mybir.AluOpType.add,
                            mybir.AluOpType.add,
                        )
                        nc.scalar.activation(
                            ow[:, t0 : t0 + nt, :],
                            ow[:, t0 : t0 + nt, :],
                            mybir.ActivationFunctionType.Relu,
                        )
                getattr(nc, STORE_ENG).dma_start(
                    out_d.ap()[r, c0 : c0 + clen, :].rearrange(
                        "(n p) c -> p n c", p=P
                    ),
                    ow[:, :nblk, :],
                )

            # Chunk list: 1024-l chunks, with the global first/last split
            # small so the pipeline fills and drains quickly.
            chunks = []
            for r in range(B_PER_CORE):
                sizes = [CHUNK] * (L // CHUNK)
                if r == 0:
                    sizes = [CHUNK // 4, CHUNK // 4, CHUNK // 2] + sizes[1:]
                if r == B_PER_CORE - 1:
                    sizes = sizes[:-1] + [CHUNK // 2, CHUNK // 4, CHUNK // 8, CHUNK // 8]
                c0 = 0
                for s in sizes:
                    chunks.append((r, c0, s))
                    c0 += s

            def new_strips(r, c0, clen):
                sa = stripa_pool.tile([P, 2, SW], fp8, tag="sa")
                se = stripe_pool.tile([P, 2, SW], fp8, tag="se")
                # SAME-padding zeros at row edges
                if c0 == 0:
                    nc.gpsimd.memset(sa[:, :, 0:2].bitcast(u8), 0)
                    nc.gpsimd.memset(se[:, :, 0:2].bitcast(u8), 0)
                if c0 + clen == L:
                    nc.gpsimd.memset(sa[:, :, 2 + clen : 4 + clen].bitcast(u8), 0)
                    nc.gpsimd.memset(se[:, :, 2 + clen : 4 + clen].bitcast(u8), 0)
                return sa, se

            def emit_transpose(n):
                rn, cn, sn = chunks[n]
                strips[n] = new_strips(rn, cn, sn)
                prev = strips.get(n - 1) if cn != 0 else None
                prev_clen = chunks[n - 1][2] if n > 0 else 0
                transpose_chunk(*strips[n], prev, prev_clen, rn, cn, sn)

            strips = {}
            for n in range(min(LA, len(chunks))):
                emit_transpose(n)
            for _k in range(KW):
                setup_weights_tap(_k)
            bias_bc2, ones_q, brow_q = setup_bias()
            for n in range(len(chunks)):
                if n + LA < len(chunks):
                    emit_transpose(n + LA)
                matmul_chunk(*strips[n], *chunks[n])
                del strips[n]

    nc.compile()
    return nc


def _get_nc():
    if "nc" not in _CACHE:
        _CACHE["nc"] = _build()
    return _CACHE["nc"]


def kernel(x: np.ndarray, W: np.ndarray, b: np.ndarray) -> np.ndarray:
    from concourse import bass_utils

    nc = _get_nc()
    x = np.ascontiguousarray(x, dtype=np.float32)
    W = np.ascontiguousarray(W, dtype=np.float32)
    b2 = np.ascontiguousarray(b, dtype=np.float32).reshape(1, COUT)
    in_maps = [
        {
            "x": x[i * B_PER_CORE : (i + 1) * B_PER_CORE],
            "W": W,
            "b": b2,
        }
        for i in range(N_CORES)
    ]
    res = bass_utils.run_bass_kernel_spmd(nc, in_maps, core_ids=list(range(N_CORES)))
    return np.concatenate([res.results[i]["out"] for i in range(N_CORES)], axis=0)
